# revision 1
# baseline (speedup 1.0000x reference)
"""Self-contained Trainium2 Bass kernel for nn_DbrxBlock_40492951667588.

DBRX block: LN1 -> GQA attention (RoPE, causal) -> residual+LN2 -> top-2/8 MoE.
8 NeuronCores, two SPMD launches:
  launch 1: token-parallel attention (core r owns batch-0 block r + batch-1
            block 7-r; causal kv sets balance to 1152 tokens/core).
  host:     router softmax/top-2 from device logits, capacity-padded dispatch.
  launch 2: expert-parallel MoE (core e owns expert e).
Matmuls run in float32r (TF32-like, ~1.5e-4 rel err); LN weights are folded
into adjacent matmul weights on the host (exact).
"""
import numpy as np
import concourse.bacc as bacc
import concourse.bass as bass
import concourse.mybir as mybir
import concourse.tile as tile
from concourse.bass_utils import run_bass_kernel_spmd

F32 = mybir.dt.float32
F32R = mybir.dt.float32r
AF = mybir.ActivationFunctionType

D = 2048
DT = D // 128          # 16 d-tiles
TKV = 1152             # kv tokens per core
NKT = TKV // 128       # 9 kv tiles
TQ = 256               # own q tokens
NH, KVH, HD = 16, 4, 128
NQB = 2
EPS = 1e-5
NEG = -30000.0

SCH = [(0, 384), (384, 384), (768, 384)]   # TKV chunks (psum-bank sized)


def bc_ap(ap, parts, n):
    """Partition-broadcast read AP: [parts, n] from a [1, n] row."""
    return bass.AP(tensor=ap.tensor, offset=ap.offset, ap=[[0, parts], [1, n]])


def build_attn(n_cores=8):
    nc = bacc.Bacc("TRN2", target_bir_lowering=False, debug=False,
                   num_devices=n_cores)
    xt = nc.dram_tensor("xt", [DT, 128, TKV], F32R, kind="ExternalInput").ap()
    wk = nc.dram_tensor("wk", [KVH, 128, DT, 128], F32R, kind="ExternalInput").ap()
    wv = nc.dram_tensor("wv", [128, DT, 512], F32R, kind="ExternalInput").ap()
    wq = nc.dram_tensor("wq", [NH, 128, DT, 128], F32R, kind="ExternalInput").ap()
    wo = nc.dram_tensor("wo", [DT, 128, DT, 128], F32R, kind="ExternalInput").ap()
    wr = nc.dram_tensor("wr", [128, DT, 8], F32R, kind="ExternalInput").ap()
    wksum = nc.dram_tensor("wksum", [128, KVH], F32, kind="ExternalInput").ap()
    wqsum = nc.dram_tensor("wqsum", [128, NH], F32, kind="ExternalInput").ap()
    wvsum = nc.dram_tensor("wvsum", [1, 512], F32, kind="ExternalInput").ap()
    cosk = nc.dram_tensor("cosk", [128, TKV], F32, kind="ExternalInput").ap()
    sink = nc.dram_tensor("sink", [128, TKV], F32, kind="ExternalInput").ap()
    cosq = nc.dram_tensor("cosq", [128, TQ], F32, kind="ExternalInput").ap()
    sinq = nc.dram_tensor("sinq", [128, TQ], F32, kind="ExternalInput").ap()
    masks = nc.dram_tensor("masks", [NQB, 128, TKV], F32, kind="ExternalInput").ap()
    ones = nc.dram_tensor("ones", [128, 1], F32R, kind="ExternalInput").ap()
    ident = nc.dram_tensor("ident", [128, 128], F32R, kind="ExternalInput").ap()

    rest = nc.dram_tensor("rest", [DT, 128, TQ], F32, kind="ExternalOutput").ap()
    h2t = nc.dram_tensor("h2t", [DT, 128, TQ], F32, kind="ExternalOutput").ap()
    logt = nc.dram_tensor("logt", [8, TQ], F32, kind="ExternalOutput").ap()

    scratch = nc.dram_tensor("scratch", [4, TKV], F32).ap()  # stat-row bounce

    with tile.TileContext(nc) as tc:
        with (
            tc.tile_pool(name="rows", bufs=1) as rows,
            tc.tile_pool(name="kvq", bufs=1) as kvq,
        ):
            ones_sb = rows.tile([128, 1], F32R)
            nc.sync.dma_start(out=ones_sb[:], in_=ones[:])
            ident_sb = rows.tile([128, 128], F32R)
            nc.sync.dma_start(out=ident_sb[:], in_=ident[:])
            wksum_sb = rows.tile([128, KVH], F32)
            nc.sync.dma_start(out=wksum_sb[:], in_=wksum[:])
            wqsum_sb = rows.tile([128, NH], F32)
            nc.sync.dma_start(out=wqsum_sb[:], in_=wqsum[:])
            wvsum_bc = rows.tile([128, 512], F32)
            nc.sync.dma_start(out=wvsum_bc[:], in_=bc_ap(wvsum, 128, 512))
            eps_t = rows.tile([1, 1], F32)
            nc.vector.memset(eps_t[:], EPS)

            kT = kvq.tile([128, KVH, TKV], F32R)
            vN = kvq.tile([128, NKT, 512], F32R)
            qT = kvq.tile([128, NH, TQ], F32R)
            xq_res = kvq.tile([128, DT, TQ], F32)

            with tc.tile_pool(name="norm", bufs=1) as norm:
                rstd_bc = norm.tile([128, TKV], F32)
                nmr_bc = norm.tile([128, TKV], F32)
                rstd_col = norm.tile([128, NKT], F32)
                nmr_col = norm.tile([128, NKT], F32)

                with tc.tile_pool(name="xp", bufs=1) as xp:
                    xts = xp.tile([128, DT, TKV], F32R)
                    for d in range(DT):
                        nc.sync.dma_start(out=xts[:, d, :], in_=xt[d])
                    xtf = xts[:].bitcast(F32)

                    # ---------------- LN1 stats ----------------
                    with (
                        tc.tile_pool(name="strow", bufs=1) as strow,
                        tc.tile_pool(name="sqp", bufs=2) as sqp,
                        tc.tile_pool(name="ps_st", bufs=1, space="PSUM") as ps_st,
                    ):
                        mu_row = strow.tile([1, TKV], F32)
                        sqm_row = strow.tile([1, TKV], F32)
                        t_row = strow.tile([1, TKV], F32)
                        psum_s = [ps_st.tile([1, w], F32, name=f"pss{i}",
                                             tag=f"pss{i}")
                                  for i, (_, w) in enumerate(SCH)]
                        psum_q = [ps_st.tile([1, w], F32, name=f"psq{i}",
                                             tag=f"psq{i}")
                                  for i, (_, w) in enumerate(SCH)]
                        for d in range(DT):
                            sq = sqp.tile([128, TKV], F32R, tag="sq")
                            nc.scalar.activation(sq[:], xtf[:, d, :], AF.Square)
                            for i, (c0, w) in enumerate(SCH):
                                nc.tensor.matmul(psum_s[i][:], ones_sb[:],
                                                 xts[:, d, c0:c0 + w],
                                                 start=(d == 0),
                                                 stop=(d == DT - 1))
                                nc.tensor.matmul(psum_q[i][:], ones_sb[:],
                                                 sq[:, c0:c0 + w],
                                                 start=(d == 0),
                                                 stop=(d == DT - 1))
                        for i, (c0, w) in enumerate(SCH):
                            nc.scalar.mul(mu_row[:, c0:c0 + w], psum_s[i][:],
                                          1.0 / D)
                            nc.scalar.mul(sqm_row[:, c0:c0 + w], psum_q[i][:],
                                          1.0 / D)
                        # var = E[x^2]-mu^2; rstd=1/sqrt(var+eps); nmr=-mu*rstd
                        nc.vector.tensor_mul(t_row[:], mu_row[:], mu_row[:])
                        nc.vector.tensor_sub(sqm_row[:], sqm_row[:], t_row[:])
                        nc.scalar.activation(sqm_row[:], sqm_row[:], AF.Sqrt,
                                             bias=eps_t[:])
                        nc.vector.reciprocal(sqm_row[:], sqm_row[:])
                        nc.vector.tensor_mul(t_row[:], mu_row[:], sqm_row[:])
                        nc.scalar.mul(t_row[:], t_row[:], -1.0)
                        nc.sync.dma_start(out=scratch[0:1, :], in_=sqm_row[:])
                        nc.sync.dma_start(out=scratch[1:2, :], in_=t_row[:])
                        nc.sync.dma_start(out=rstd_bc[:],
                                          in_=bc_ap(scratch[0:1, :], 128, TKV))
                        nc.sync.dma_start(out=nmr_bc[:],
                                          in_=bc_ap(scratch[1:2, :], 128, TKV))
                        nc.sync.dma_start(
                            out=rstd_col[:],
                            in_=scratch[0, :].rearrange("(t p) -> p t", p=128))
                        nc.sync.dma_start(
                            out=nmr_col[:],
                            in_=scratch[1, :].rearrange("(t p) -> p t", p=128))

                    # ---------------- K proj + rope ----------------
                    with (
                        tc.tile_pool(name="ckp", bufs=1) as ckp,
                        tc.tile_pool(name="wkp", bufs=2) as wkp,
                        tc.tile_pool(name="ktp", bufs=2) as ktp,
                        tc.tile_pool(name="kf1", bufs=2) as kf1,
                        tc.tile_pool(name="ps_k", bufs=2, space="PSUM") as ps_k,
                    ):
                        cosk_sb = ckp.tile([128, TKV], F32)
                        nc.sync.dma_start(out=cosk_sb[:], in_=cosk[:])
                        sink_sb = ckp.tile([128, TKV], F32)
                        nc.sync.dma_start(out=sink_sb[:], in_=sink[:])
                        for ok in range(KVH):
                            wk_sb = wkp.tile([128, DT, 128], F32R, tag="wk")
                            nc.sync.dma_start(out=wk_sb[:], in_=wk[ok])
                            psk = [ps_k.tile([128, w], F32, name=f"psk{i}",
                                             tag=f"psk{i}")
                                   for i, (_, w) in enumerate(SCH)]
                            for d in range(DT):
                                for i, (c0, w) in enumerate(SCH):
                                    nc.tensor.matmul(psk[i][:], wk_sb[:, d, :],
                                                     xts[:, d, c0:c0 + w],
                                                     start=(d == 0),
                                                     stop=(d == DT - 1))
                            ktmp = ktp.tile([128, TKV], F32, tag="ktmp")
                            krot = ktp.tile([128, TKV], F32, tag="krot")
                            for i, (c0, w) in enumerate(SCH):
                                t1 = kf1.tile([128, 384], F32, tag="kpf1")
                                nc.scalar.activation(
                                    t1[:, :w], nmr_bc[:, c0:c0 + w], AF.Copy,
                                    scale=wksum_sb[:, ok:ok + 1])
                                nc.vector.tensor_mul(ktmp[:, c0:c0 + w],
                                                     psk[i][:],
                                                     rstd_bc[:, c0:c0 + w])
                                nc.vector.tensor_add(ktmp[:, c0:c0 + w],
                                                     ktmp[:, c0:c0 + w],
                                                     t1[:, :w])
                            nc.sync.dma_start(out=krot[0:64, :],
                                              in_=ktmp[64:128, :])
                            nc.sync.dma_start(out=krot[64:128, :],
                                              in_=ktmp[0:64, :])
                            nc.vector.tensor_mul(ktmp[:], ktmp[:], cosk_sb[:])
                            nc.vector.tensor_mul(krot[:], krot[:], sink_sb[:])
                            nc.vector.tensor_add(kT[:, ok, :], ktmp[:], krot[:])

                    # ---------------- V proj (t-major) ----------------
                    with (
                        tc.tile_pool(name="wvp", bufs=1) as wvp,
                        tc.tile_pool(name="vf1", bufs=2) as vf1,
                        tc.tile_pool(name="ps_v", bufs=2, space="PSUM") as ps_v,
                    ):
                        wv_sb = wvp.tile([128, DT, 512], F32R)
                        nc.sync.dma_start(out=wv_sb[:], in_=wv[:])
                        for tv in range(NKT):
                            psv = ps_v.tile([128, 512], F32, tag="psv")
                            for d in range(DT):
                                nc.tensor.matmul(
                                    psv[:], xts[:, d, tv * 128:(tv + 1) * 128],
                                    wv_sb[:, d, :],
                                    start=(d == 0), stop=(d == DT - 1))
                            t1 = vf1.tile([128, 512], F32, tag="vpf1")
                            nc.scalar.activation(t1[:], wvsum_bc[:], AF.Copy,
                                                 scale=nmr_col[:, tv:tv + 1])
                            t2 = vf1.tile([128, 512], F32, tag="vpf2")
                            nc.vector.tensor_scalar_mul(
                                t2[:], in0=psv[:],
                                scalar1=rstd_col[:, tv:tv + 1])
                            nc.vector.tensor_add(vN[:, tv, :], t1[:], t2[:])

                    # ---------------- Q proj + rope ----------------
                    with (
                        tc.tile_pool(name="cqp", bufs=1) as cqp,
                        tc.tile_pool(name="wqp", bufs=3) as wqp,
                        tc.tile_pool(name="qtp", bufs=2) as qtp,
                        tc.tile_pool(name="ps_q", bufs=2, space="PSUM") as ps_q,
                    ):
                        cosq_sb = cqp.tile([128, TQ], F32)
                        nc.sync.dma_start(out=cosq_sb[:], in_=cosq[:])
                        sinq_sb = cqp.tile([128, TQ], F32)
                        nc.sync.dma_start(out=sinq_sb[:], in_=sinq[:])
                        for oq in range(NH):
                            wq_sb = wqp.tile([128, DT, 128], F32R, tag="wq")
                            nc.sync.dma_start(out=wq_sb[:], in_=wq[oq])
                            psq = ps_q.tile([128, TQ], F32, tag="psq")
                            for d in range(DT):
                                nc.tensor.matmul(psq[:], wq_sb[:, d, :],
                                                 xts[:, d, 0:TQ],
                                                 start=(d == 0),
                                                 stop=(d == DT - 1))
                            qtmp = qtp.tile([128, TQ], F32, tag="qtmp")
                            qrot = qtp.tile([128, TQ], F32, tag="qrot")
                            t1 = qtp.tile([128, TQ], F32, tag="qpf1")
                            nc.scalar.activation(t1[:], nmr_bc[:, 0:TQ],
                                                 AF.Copy,
                                                 scale=wqsum_sb[:, oq:oq + 1])
                            nc.vector.tensor_mul(qtmp[:], psq[:],
                                                 rstd_bc[:, 0:TQ])
                            nc.vector.tensor_add(qtmp[:], qtmp[:], t1[:])
                            nc.sync.dma_start(out=qrot[0:64, :],
                                              in_=qtmp[64:128, :])
                            nc.sync.dma_start(out=qrot[64:128, :],
                                              in_=qtmp[0:64, :])
                            nc.vector.tensor_mul(qtmp[:], qtmp[:], cosq_sb[:])
                            nc.vector.tensor_mul(qrot[:], qrot[:], sinq_sb[:])
                            nc.vector.tensor_add(qT[:, oq, :], qtmp[:], qrot[:])

                    # own-q raw x for the residual add (outlives xts)
                    nc.vector.tensor_copy(xq_res[:], xtf[:, :, 0:TQ])

            # ---------------- attention ----------------
            with tc.tile_pool(name="attp", bufs=1) as attp:
                attnT = attp.tile([128, NH, TQ], F32R)
                with (
                    tc.tile_pool(name="mkp", bufs=1) as mkp,
                    tc.tile_pool(name="scp", bufs=2) as scp,
                    tc.tile_pool(name="srp", bufs=2) as srp,
                    tc.tile_pool(name="ptsp", bufs=2) as ptsp,
                    tc.tile_pool(name="ps_s", bufs=1, space="PSUM") as ps_s,
                    tc.tile_pool(name="ps_t", bufs=2, space="PSUM") as ps_t,
                    tc.tile_pool(name="ps_a", bufs=2, space="PSUM") as ps_a,
                ):
                    mask_sb = mkp.tile([128, NQB, TKV], F32)
                    nc.sync.dma_start(out=mask_sb[:],
                                      in_=masks.rearrange("b p t -> p b t"))
                    for kvh in range(KVH):
                        for qb in range(NQB):
                            pns = []
                            for j in range(4):
                                h = kvh * 4 + j
                                s_sb = scp.tile([128, TKV], F32, tag=f"s{j}")
                                rs = srp.tile([128, 2], F32, tag=f"rs{j}")
                                for i, (c0, w) in enumerate(SCH):
                                    pss = ps_s.tile([128, w], F32,
                                                    name=f"pssc{i}",
                                                    tag=f"pssc{i}")
                                    nc.tensor.matmul(
                                        pss[:],
                                        qT[:, h, qb * 128:(qb + 1) * 128],
                                        kT[:, kvh, c0:c0 + w])
                                    nc.vector.tensor_add(
                                        s_sb[:, c0:c0 + w], pss[:],
                                        mask_sb[:, qb, c0:c0 + w])
                                nc.scalar.activation(s_sb[:], s_sb[:], AF.Exp,
                                                     accum_out=rs[:, 0:1])
                                nc.vector.reciprocal(rs[:, 1:2], rs[:, 0:1])
                                pn = scp.tile([128, TKV], F32R, tag=f"pn{j}")
                                nc.vector.tensor_scalar_mul(
                                    pn[:], in0=s_sb[:], scalar1=rs[:, 1:2])
                                pns.append(pn)
                            psa = ps_a.tile([128, 512], F32, tag="psa")
                            for kt in range(NKT):
                                ptp = ps_t.tile([128, 512], F32R, tag="ptp")
                                for j in range(4):
                                    nc.tensor.transpose(
                                        ptp[:, j * 128:(j + 1) * 128],
                                        pns[j][:, kt * 128:(kt + 1) * 128],
                                        ident_sb[:])
                                pts = ptsp.tile([128, 512], F32R, tag="pts")
                                nc.scalar.copy(pts[:], ptp[:].bitcast(F32))
                                nc.tensor.matmul(
                                    psa[:],
                                    vN[:, kt, kvh * 128:(kvh + 1) * 128],
                                    pts[:],
                                    start=(kt == 0), stop=(kt == NKT - 1))
                            nc.scalar.copy(
                                attnT[:, kvh * 4:(kvh + 1) * 4,
                                      qb * 128:(qb + 1) * 128],
                                psa[:].rearrange("p (j q) -> p j q", j=4))

                # ---------------- out-proj + residual + LN2 ----------------
                with (
                    tc.tile_pool(name="outp", bufs=1) as outp,
                    tc.tile_pool(name="wop", bufs=3) as wop,
                    tc.tile_pool(name="sq2p", bufs=2) as sq2p,
                    tc.tile_pool(name="ps_o", bufs=2, space="PSUM") as ps_o,
                    tc.tile_pool(name="ps_l2", bufs=1, space="PSUM") as ps_l2,
                ):
                    residT = outp.tile([128, DT, TQ], F32R)
                    h2s = outp.tile([128, DT, TQ], F32R)
                    ps2s = ps_l2.tile([1, TQ], F32, tag="ps2s")
                    ps2q = ps_l2.tile([1, TQ], F32, tag="ps2q")
                    for d2 in range(DT):
                        wo_sb = wop.tile([128, DT, 128], F32R, tag="wo")
                        nc.sync.dma_start(out=wo_sb[:], in_=wo[d2])
                        pso = ps_o.tile([128, TQ], F32, tag="pso")
                        for o in range(DT):
                            nc.tensor.matmul(pso[:], wo_sb[:, o, :],
                                             attnT[:, o, :],
                                             start=(o == 0), stop=(o == DT - 1))
                        nc.vector.tensor_add(residT[:, d2, :], pso[:],
                                             xq_res[:, d2, :])
                        nc.sync.dma_start(out=rest[d2],
                                          in_=residT[:, d2, :].bitcast(F32))
                        sq2 = sq2p.tile([128, TQ], F32R, tag="sq2")
                        nc.scalar.activation(sq2[:],
                                             residT[:, d2, :].bitcast(F32),
                                             AF.Square)
                        nc.tensor.matmul(ps2s[:], ones_sb[:], residT[:, d2, :],
                                         start=(d2 == 0), stop=(d2 == DT - 1))
                        nc.tensor.matmul(ps2q[:], ones_sb[:], sq2[:],
                                         start=(d2 == 0), stop=(d2 == DT - 1))
                    # LN2 rows
                    mu2 = outp.tile([1, TQ], F32)
                    sqm2 = outp.tile([1, TQ], F32)
                    t_r2 = outp.tile([1, TQ], F32)
                    nc.scalar.mul(mu2[:], ps2s[:], 1.0 / D)
                    nc.scalar.mul(sqm2[:], ps2q[:], 1.0 / D)
                    nc.vector.tensor_mul(t_r2[:], mu2[:], mu2[:])
                    nc.vector.tensor_sub(sqm2[:], sqm2[:], t_r2[:])
                    nc.scalar.activation(sqm2[:], sqm2[:], AF.Sqrt,
                                         bias=eps_t[:])
                    nc.vector.reciprocal(sqm2[:], sqm2[:])
                    nc.vector.tensor_mul(t_r2[:], mu2[:], sqm2[:])
                    nc.scalar.mul(t_r2[:], t_r2[:], -1.0)
                    nc.sync.dma_start(out=scratch[2:3, 0:TQ], in_=sqm2[:])
                    nc.sync.dma_start(out=scratch[3:4, 0:TQ], in_=t_r2[:])
                    rstd2_bc = outp.tile([128, TQ], F32)
                    nc.sync.dma_start(out=rstd2_bc[:],
                                      in_=bc_ap(scratch[2:3, 0:TQ], 128, TQ))
                    nmr2_bc = outp.tile([128, TQ], F32)
                    nc.sync.dma_start(out=nmr2_bc[:],
                                      in_=bc_ap(scratch[3:4, 0:TQ], 128, TQ))

                    # ---------------- h2 + router logits ----------------
                    with (
                        tc.tile_pool(name="wrp", bufs=1) as wrp,
                        tc.tile_pool(name="ps_r", bufs=1, space="PSUM") as ps_r,
                    ):
                        wr_sb = wrp.tile([128, DT, 8], F32R)
                        nc.sync.dma_start(out=wr_sb[:], in_=wr[:])
                        psl = ps_r.tile([8, TQ], F32, tag="psl")
                        for d2 in range(DT):
                            nc.vector.tensor_mul(h2s[:, d2, :],
                                                 residT[:, d2, :].bitcast(F32),
                                                 rstd2_bc[:])
                            nc.vector.tensor_add(h2s[:, d2, :],
                                                 h2s[:, d2, :].bitcast(F32),
                                                 nmr2_bc[:])
                            nc.sync.dma_start(out=h2t[d2],
                                              in_=h2s[:, d2, :].bitcast(F32))
                            nc.tensor.matmul(psl[:], wr_sb[:, d2, :],
                                             h2s[:, d2, :],
                                             start=(d2 == 0),
                                             stop=(d2 == DT - 1))
                        lo = outp.tile([8, TQ], F32)
                        nc.scalar.copy(lo[:], psl[:])
                        nc.sync.dma_start(out=logt[:], in_=lo[:])
    nc.compile()
    return nc


# ======================= host-side prep =======================

def core_colmap(r, NB=8, BLK=128):
    """(batch, pos) per column for core r. cols: [own qb0, own qb1, rest]."""
    b = []
    b += [(0, r * BLK + i) for i in range(BLK)]
    b += [(1, (NB - 1 - r) * BLK + i) for i in range(BLK)]
    for j in range(r):
        b += [(0, j * BLK + i) for i in range(BLK)]
    for j in range(NB - 1 - r):
        b += [(1, j * BLK + i) for i in range(BLK)]
    return b


def host_attn_inputs(x, cos, sin, ln1_w, w_qkv, w_out, w_router, ln2_w,
                     n_cores=8):
    """Per-core input maps for build_attn. x [B,S,D]; cos/sin [S,HD]."""
    B, S, Dm = x.shape
    NB, BLK = S // 128, 128
    wqkvT = (w_qkv * ln1_w[None, :]).T.astype(np.float32)      # [D, 3072]
    wqm = wqkvT[:, :NH * HD]                                    # [D, 2048] Q
    wkm = wqkvT[:, NH * HD:NH * HD + 512]                       # [D, 512] K
    wvm = wqkvT[:, NH * HD + 512:]                              # [D, 512] V
    w_outT = w_out.T.astype(np.float32)                         # [O, D]
    sinp = sin.copy()
    sinp[:, :HD // 2] *= -1.0
    scale = np.float32(1.0 / np.sqrt(HD))

    wk_in = np.ascontiguousarray(
        wkm.reshape(DT, 128, KVH, 128).transpose(2, 1, 0, 3))  # [ok, p, d, k]
    wv_in = np.ascontiguousarray(wvm.reshape(DT, 128, 512).transpose(1, 0, 2))
    wq_in = np.ascontiguousarray(
        wqm.reshape(DT, 128, NH, 128).transpose(2, 1, 0, 3))   # [oq, p, d, k]
    wo_in = np.ascontiguousarray(
        w_outT.reshape(DT, 128, DT, 128).transpose(2, 1, 0, 3))  # [d2, p, o, k]
    wr_in = np.ascontiguousarray(
        ((w_router * ln2_w[None, :]).T.astype(np.float32))
        .reshape(DT, 128, 8).transpose(1, 0, 2))               # [p, d, 8]
    wksum = np.ascontiguousarray(wkm.sum(0).reshape(KVH, 128).T)  # [128, KVH]
    wqsum = np.ascontiguousarray(wqm.sum(0).reshape(NH, 128).T)   # [128, NH]
    wvsum = np.ascontiguousarray(wvm.sum(0).reshape(1, 512))
    ident = np.eye(128, dtype=np.float32)
    ones_in = np.ones((128, 1), np.float32)

    maps = []
    for r in range(n_cores):
        cm = core_colmap(r, NB, BLK)
        bs = np.array([c[0] for c in cm])
        ps = np.array([c[1] for c in cm])
        xTc = np.ascontiguousarray(x[bs, ps, :].T)              # [D, TKV]
        ck = np.ascontiguousarray(cos[ps].T)                    # [HD, TKV]
        sk = np.ascontiguousarray(sinp[ps].T)
        cq = np.ascontiguousarray(cos[ps[:TQ]].T) * scale
        sq = np.ascontiguousarray(sinp[ps[:TQ]].T) * scale
        msk = np.full((NQB, 128, TKV), NEG, np.float32)
        for qb in range(NQB):
            qb_b = bs[qb * 128]
            qb_p = ps[qb * 128:(qb + 1) * 128]
            okm = (bs[None, :] == qb_b) & (ps[None, :] <= qb_p[:, None])
            msk[qb][okm] = 0.0
        maps.append({
            "xt": np.ascontiguousarray(xTc.reshape(DT, 128, TKV)),
            "wk": wk_in, "wv": wv_in, "wq": wq_in, "wo": wo_in, "wr": wr_in,
            "wksum": wksum, "wqsum": wqsum, "wvsum": wvsum,
            "cosk": ck, "sink": sk, "cosq": cq, "sinq": sq,
            "masks": msk, "ones": ones_in, "ident": ident,
        })
    return maps


def assemble_attn_outputs(results, n_cores=8, NB=8, BLK=128):
    """results: per-core dicts. Returns h2T_full [D,T], resid_full [D,T],
    logits [T, 8] in (batch, pos) token order."""
    T = 2 * NB * BLK
    h2T = np.zeros((D, T), np.float32)
    rT = np.zeros((D, T), np.float32)
    lg = np.zeros((T, 8), np.float32)
    for r in range(n_cores):
        cm = core_colmap(r, NB, BLK)
        toks = np.array([b * NB * BLK + p for b, p in cm[:TQ]])
        h2T[:, toks] = results[r]["h2t"].reshape(D, TQ)
        rT[:, toks] = results[r]["rest"].reshape(D, TQ)
        lg[toks] = results[r]["logt"].T
    return h2T, rT, lg

# ======================= MoE launch (expert parallel) =======================
MD, MF = 2048, 2048
DT_, FT = MD // 128, MF // 128

def chunks(C):
    # free-dim chunks <=512 (PSUM bank), prefer fewest chunks all >=256
    if C <= 512:
        return [(0, C)]
    if C <= 1024:
        h = (C // 2 + 31) // 32 * 32
        return [(0, h), (h, C - h)]
    return [(0, 512), (512, 512), (1024, C - 1024)]


def build_moe(C, n_cores=8):
    CH = chunks(C)
    nc = bacc.Bacc("TRN2", target_bir_lowering=False, debug=False,
                   num_devices=n_cores)
    xe = nc.dram_tensor("xe", [DT_, 128, C], F32R, kind="ExternalInput").ap()
    wg = nc.dram_tensor("wg", [FT, 128, DT_, 128], F32R, kind="ExternalInput").ap()
    wu = nc.dram_tensor("wu", [FT, 128, DT_, 128], F32R, kind="ExternalInput").ap()
    wd = nc.dram_tensor("wd", [DT_, 128, FT, 128], F32R, kind="ExternalInput").ap()
    wec = nc.dram_tensor("wec", [1, C], F32, kind="ExternalInput").ap()
    ye = nc.dram_tensor("ye", [DT_, 128, C], F32, kind="ExternalOutput").ap()

    with tile.TileContext(nc) as tc:
        with (
            tc.tile_pool(name="res", bufs=1) as res,
            tc.tile_pool(name="wp", bufs=3) as wp,
            tc.tile_pool(name="sg", bufs=3) as sgp,
            tc.tile_pool(name="yo", bufs=3) as yop,
        ):
            xsb = res.tile([128, DT_, C], F32R)
            for d in range(DT_):
                nc.sync.dma_start(out=xsb[:, d, :], in_=xe[d])
            webc = res.tile([128, C], F32)
            nc.sync.dma_start(
                out=webc[:],
                in_=bass.AP(tensor=wec.tensor, offset=wec.offset,
                            ap=[[0, 128], [1, C]]),
            )
            mT = res.tile([128, FT, C], F32R)

            # --- gate/up + silu*u -> mT ---
            with (
                tc.tile_pool(name="psgu", bufs=1, space="PSUM") as psg,
                tc.tile_pool(name="psy", bufs=2, space="PSUM") as psy,
            ):
                for f in range(FT):
                    pgs = [psg.tile([128, w], F32, name=f"pg{ci}", tag=f"pg{ci}")
                           for ci, (_, w) in enumerate(CH)]
                    pus = [psg.tile([128, w], F32, name=f"pu{ci}", tag=f"pu{ci}")
                           for ci, (_, w) in enumerate(CH)]
                    wgt = wp.tile([128, DT_, 128], F32R, tag="wg")
                    nc.sync.dma_start(out=wgt[:], in_=wg[f])
                    wut = wp.tile([128, DT_, 128], F32R, tag="wu")
                    nc.sync.dma_start(out=wut[:], in_=wu[f])
                    for d in range(DT_):
                        for ci, (c0, w) in enumerate(CH):
                            nc.tensor.matmul(pgs[ci][:], wgt[:, d, :],
                                             xsb[:, d, c0:c0 + w],
                                             start=(d == 0), stop=(d == DT_ - 1))
                        for ci, (c0, w) in enumerate(CH):
                            nc.tensor.matmul(pus[ci][:], wut[:, d, :],
                                             xsb[:, d, c0:c0 + w],
                                             start=(d == 0), stop=(d == DT_ - 1))
                    for ci, (c0, w) in enumerate(CH):
                        sg = sgp.tile([128, 512], F32, tag="sg")
                        nc.scalar.activation(sg[:, :w], pgs[ci][:],
                                             mybir.ActivationFunctionType.Silu)
                        nc.vector.tensor_mul(mT[:, f, c0:c0 + w], sg[:, :w],
                                             pus[ci][:])

                # --- down + combine-weight scale -> ye ---
                for d2 in range(DT_):
                    pys = [psy.tile([128, w], F32, name=f"py{ci}", tag=f"py{ci}")
                           for ci, (_, w) in enumerate(CH)]
                    wdt = wp.tile([128, FT, 128], F32R, tag="wd")
                    nc.sync.dma_start(out=wdt[:], in_=wd[d2])
                    for f in range(FT):
                        for ci, (c0, w) in enumerate(CH):
                            nc.tensor.matmul(pys[ci][:], wdt[:, f, :],
                                             mT[:, f, c0:c0 + w],
                                             start=(f == 0), stop=(f == FT - 1))
                    for ci, (c0, w) in enumerate(CH):
                        yt = yop.tile([128, 512], F32, tag="yt")
                        nc.vector.tensor_mul(yt[:, :w], pys[ci][:],
                                             webc[:, c0:c0 + w])
                        nc.sync.dma_start(out=ye[d2, :, c0:c0 + w], in_=yt[:, :w])
    nc.compile()
    return nc


def host_moe_inputs(h2T_full, assign, aw, C, w_gate_f, w_up_f, w_down):
    """Build per-core input maps. h2T_full [D, T]; assign/aw lists per expert."""
    E = len(assign)
    maps = []
    for e in range(E):
        n = len(assign[e])
        assert n <= C, f"expert {e} count {n} > capacity {C}"
        xeT = np.zeros((MD, C), np.float32)
        xeT[:, :n] = h2T_full[:, assign[e]]
        wec = np.zeros((1, C), np.float32)
        wec[0, :n] = aw[e]
        maps.append({
            "xe": np.ascontiguousarray(xeT.reshape(DT_, 128, C)),
            "wg": np.ascontiguousarray(
                w_gate_f[e].reshape(DT_, 128, FT, 128).transpose(2, 1, 0, 3)),
            "wu": np.ascontiguousarray(
                w_up_f[e].reshape(DT_, 128, FT, 128).transpose(2, 1, 0, 3)),
            "wd": np.ascontiguousarray(
                w_down[e].reshape(FT, 128, DT_, 128).transpose(2, 1, 0, 3)),
            "wec": wec,
        })
    return maps


# ======================= top-level kernel =======================
E, K_TOP = 8, 2
_cache = {}


def _routing(logits):
    lm = logits.max(1, keepdims=True)
    p = np.exp(logits - lm)
    p /= p.sum(1, keepdims=True)
    top_e = np.argsort(-p, 1)[:, :K_TOP]
    top_w = np.take_along_axis(p, top_e, 1)
    top_w = top_w / np.abs(top_w).sum(1, keepdims=True)
    flat_e = top_e.ravel()
    flat_t = np.repeat(np.arange(logits.shape[0]), K_TOP)
    flat_w = top_w.ravel()
    assign = [flat_t[flat_e == e] for e in range(E)]
    aw = [flat_w[flat_e == e] for e in range(E)]
    return assign, aw


def kernel(hidden_states, cos, sin, ln1_w, ln2_w, w_qkv, w_out,
           w_router, w_gate, w_up, w_down):
    hidden_states = np.asarray(hidden_states, np.float32)
    cos = np.asarray(cos, np.float32)
    sin = np.asarray(sin, np.float32)
    ln1_w = np.asarray(ln1_w, np.float32)
    ln2_w = np.asarray(ln2_w, np.float32)
    w_qkv = np.asarray(w_qkv, np.float32)
    w_out = np.asarray(w_out, np.float32)
    w_router = np.asarray(w_router, np.float32)
    w_gate = np.asarray(w_gate, np.float32)
    w_up = np.asarray(w_up, np.float32)
    w_down = np.asarray(w_down, np.float32)
    B, S, Dm = hidden_states.shape

    if "attn" not in _cache:
        _cache["attn"] = build_attn()
    maps = host_attn_inputs(hidden_states, cos, sin, ln1_w, w_qkv, w_out,
                            w_router, ln2_w)
    res1 = run_bass_kernel_spmd(_cache["attn"], maps, list(range(8)))
    h2T, rT, lg = assemble_attn_outputs(res1.results)

    assign, aw = _routing(lg)
    counts = [len(a) for a in assign]
    C = max(256, (max(counts) + 63) // 64 * 64)

    if ("moe", C) not in _cache:
        _cache[("moe", C)] = build_moe(C)
    w_gate_f = w_gate * ln2_w[None, :, None]
    w_up_f = w_up * ln2_w[None, :, None]
    maps2 = host_moe_inputs(h2T, assign, aw, C, w_gate_f, w_up_f, w_down)
    res2 = run_bass_kernel_spmd(_cache[("moe", C)], maps2, list(range(8)))

    T = B * S
    out_full = np.zeros((T, MD), np.float32)
    for e in range(E):
        ye = res2.results[e]["ye"].reshape(MD, C)
        n = counts[e]
        out_full[assign[e]] += ye[:, :n].T

    out = out_full.reshape(B, S, Dm)
    residual = rT.T.reshape(B, S, Dm)
    return out, residual



# revision 5
# speedup vs baseline: 1.7940x; 1.7940x over previous
"""Self-contained Trainium2 Bass kernel for nn_DbrxBlock_40492951667588.

DBRX block: LN1 -> GQA attention (RoPE, causal) -> residual+LN2 -> top-2/8 MoE.
8 NeuronCores, two SPMD launches, all matmuls in bf16 (fp32 psum accumulate):

  launch 1 (attention, batch x kv-head sharding): core (b, kvh) projects
    K/V for its one kv head and Q for its 4 q-heads over batch b's 1024
    tokens, computes causal scores transposed (S^T = K^T tile x Q, so no
    P transposes and no masked-out compute), AV, and a partial out-proj
    over its 4 heads' dims.  LN1 is folded in host-side (rstd/shift are
    baked into rope cos/sin and additive tiles); LN2/router run on host.
  host: reduce out-proj partials + residual, LN2, router logits, top-2.
    Tokens whose top2/top3 margin is below a threshold get their logits
    recomputed exactly (numpy fp32 attention rows) so expert selection
    always matches the fp32 reference regardless of device rounding.
  launch 2 (MoE, expert parallel): core e owns expert e, capacity-padded
    token batch, bf16 gate/up/silu/down, combine weights applied on-core.
"""
import numpy as np
import ml_dtypes
import concourse.bacc as bacc
import concourse.bass as bass
import concourse.mybir as mybir
import concourse.tile as tile
from concourse.bass_utils import run_bass_kernel_spmd

F32 = mybir.dt.float32
BF16 = mybir.dt.bfloat16
AF = mybir.ActivationFunctionType
BF = ml_dtypes.bfloat16

B, S, D = 2, 1024, 2048
H, KV, HD = 16, 4, 128
E, K_TOP, F = 8, 2, 2048
DT = D // 128            # 16 d-tiles
NB = S // 128            # 8 token blocks per batch
EPS = 1e-5
QH = H // KV             # 4 q heads per kv head / core
THETA = 0.025            # top2/top3 logit margin below which host recomputes


def bc_ap(ap, parts, n):
    """Partition-broadcast read AP: [parts, n] from a [1, n] row."""
    return bass.AP(tensor=ap.tensor, offset=ap.offset, ap=[[0, parts], [1, n]])


# ======================= attention launch =======================

def build_attn(n_cores=8):
    nc = bacc.Bacc("TRN2", target_bir_lowering=False, debug=False,
                   num_devices=n_cores)
    xt = nc.dram_tensor("xt", [DT, 128, S], BF16, kind="ExternalInput").ap()
    wk = nc.dram_tensor("wk", [128, DT, 128], BF16, kind="ExternalInput").ap()
    wv = nc.dram_tensor("wv", [128, DT, 128], BF16, kind="ExternalInput").ap()
    wq = nc.dram_tensor("wq", [QH, 128, DT, 128], BF16, kind="ExternalInput").ap()
    wo = nc.dram_tensor("wo", [QH, 128, DT, 128], BF16, kind="ExternalInput").ap()
    cosr = nc.dram_tensor("cosr", [128, S], BF16, kind="ExternalInput").ap()
    sinr = nc.dram_tensor("sinr", [128, S], BF16, kind="ExternalInput").ap()
    cosrq = nc.dram_tensor("cosrq", [128, S], BF16, kind="ExternalInput").ap()
    sinrq = nc.dram_tensor("sinrq", [128, S], BF16, kind="ExternalInput").ap()
    ak = nc.dram_tensor("ak", [128, S], BF16, kind="ExternalInput").ap()
    aq = nc.dram_tensor("aq", [QH, 128, S], BF16, kind="ExternalInput").ap()
    av = nc.dram_tensor("av", [128, S], BF16, kind="ExternalInput").ap()
    rstdr = nc.dram_tensor("rstdr", [1, S], F32, kind="ExternalInput").ap()
    tri = nc.dram_tensor("tri", [128, 128], BF16, kind="ExternalInput").ap()
    ident = nc.dram_tensor("ident", [128, 128], BF16, kind="ExternalInput").ap()
    ones = nc.dram_tensor("ones", [128, 1], BF16, kind="ExternalInput").ap()

    pout = nc.dram_tensor("pout", [DT, 128, S], BF16, kind="ExternalOutput").ap()
    scratch = nc.dram_tensor("scratch", [QH, S], F32).ap()  # denom bounce

    with tile.TileContext(nc) as tc:
        with (
            tc.tile_pool(name="cst", bufs=1) as cst,
            tc.tile_pool(name="big", bufs=1) as big,
        ):
            tri_sb = cst.tile([128, 128], BF16)
            nc.sync.dma_start(out=tri_sb[:], in_=tri[:])
            id_sb = cst.tile([128, 128], BF16)
            nc.sync.dma_start(out=id_sb[:], in_=ident[:])
            ones_sb = cst.tile([128, 1], BF16)
            nc.sync.dma_start(out=ones_sb[:], in_=ones[:])
            rstd_bc = cst.tile([128, S], F32)
            nc.sync.dma_start(out=rstd_bc[:], in_=bc_ap(rstdr, 128, S))

            xts = big.tile([128, DT, S], BF16)
            for d in range(DT):
                nc.sync.dma_start(out=xts[:, d, :], in_=xt[d])
            kT = big.tile([128, S], BF16)
            vN = big.tile([128, NB, 128], BF16)      # [t-part, kv-tile, hd]
            qT = big.tile([128, QH, S], BF16)
            avn = big.tile([128, QH, S], BF16)       # normalized AV^T per head

            # ---------------- projections + rope ----------------
            with (
                tc.tile_pool(name="cs", bufs=1) as cs,
                tc.tile_pool(name="rp", bufs=2) as rp,
                tc.tile_pool(name="ps_p", bufs=2, space="PSUM") as ps_p,
                tc.tile_pool(name="ps_t", bufs=2, space="PSUM") as ps_t,
            ):
                cosr_sb = cs.tile([128, S], BF16)
                nc.sync.dma_start(out=cosr_sb[:], in_=cosr[:])
                sinr_sb = cs.tile([128, S], BF16)
                nc.sync.dma_start(out=sinr_sb[:], in_=sinr[:])
                cosrq_sb = cs.tile([128, S], BF16)
                nc.sync.dma_start(out=cosrq_sb[:], in_=cosrq[:])
                sinrq_sb = cs.tile([128, S], BF16)
                nc.sync.dma_start(out=sinrq_sb[:], in_=sinrq[:])
                ak_sb = cs.tile([128, S], BF16)
                nc.sync.dma_start(out=ak_sb[:], in_=ak[:])
                aq_sb = cs.tile([128, QH, S], BF16)
                for j in range(QH):
                    nc.sync.dma_start(out=aq_sb[:, j, :], in_=aq[j])
                av_sb = cs.tile([128, S], BF16)
                nc.sync.dma_start(out=av_sb[:], in_=av[:])
                wk_sb = cs.tile([128, DT, 128], BF16)
                nc.sync.dma_start(out=wk_sb[:], in_=wk[:])
                wv_sb = cs.tile([128, DT, 128], BF16)
                nc.sync.dma_start(out=wv_sb[:], in_=wv[:])
                wq_sb = cs.tile([128, QH, DT, 128], BF16)
                for j in range(QH):
                    nc.sync.dma_start(out=wq_sb[:, j], in_=wq[j])

                # K: k = kraw*cosr + rot(kraw)*sinr + ak
                for c in range(2):
                    sl = slice(c * 512, (c + 1) * 512)
                    psk = ps_p.tile([128, 512], F32, tag="psp")
                    for d in range(DT):
                        nc.tensor.matmul(psk[:], wk_sb[:, d, :], xts[:, d, sl],
                                         start=(d == 0), stop=(d == DT - 1))
                    t0 = rp.tile([128, 512], F32, tag="t0")
                    t1 = rp.tile([128, 512], F32, tag="t1")
                    tr = rp.tile([128, 512], F32, tag="tr")
                    nc.vector.tensor_mul(t0[:], psk[:], cosr_sb[:, sl])
                    nc.vector.tensor_mul(t1[:], psk[:], sinr_sb[:, sl])
                    nc.sync.dma_start(out=tr[0:64, :], in_=t1[64:128, :])
                    nc.sync.dma_start(out=tr[64:128, :], in_=t1[0:64, :])
                    nc.vector.tensor_add(t0[:], t0[:], ak_sb[:, sl])
                    nc.vector.tensor_add(kT[:, sl], t0[:], tr[:])

                # V^T then transpose to [t, hd]
                vtt = cs.tile([128, S], BF16)
                for c in range(2):
                    sl = slice(c * 512, (c + 1) * 512)
                    psv = ps_p.tile([128, 512], F32, tag="psp")
                    for d in range(DT):
                        nc.tensor.matmul(psv[:], wv_sb[:, d, :], xts[:, d, sl],
                                         start=(d == 0), stop=(d == DT - 1))
                    t0 = rp.tile([128, 512], F32, tag="t0")
                    nc.vector.tensor_mul(t0[:], psv[:], rstd_bc[:, sl])
                    nc.vector.tensor_add(vtt[:, sl], t0[:], av_sb[:, sl])
                for tt in range(NB):
                    ptp = ps_t.tile([128, 128], BF16, tag="ptp")
                    nc.tensor.transpose(ptp[:], vtt[:, tt * 128:(tt + 1) * 128],
                                        id_sb[:])
                    nc.scalar.copy(vN[:, tt, :], ptp[:])

                # Q (scale 1/sqrt(HD) folded into cosrq/sinrq/aq)
                for j in range(QH):
                    for c in range(2):
                        sl = slice(c * 512, (c + 1) * 512)
                        psq = ps_p.tile([128, 512], F32, tag="psp")
                        for d in range(DT):
                            nc.tensor.matmul(psq[:], wq_sb[:, j, d, :],
                                             xts[:, d, sl],
                                             start=(d == 0), stop=(d == DT - 1))
                        t0 = rp.tile([128, 512], F32, tag="t0")
                        t1 = rp.tile([128, 512], F32, tag="t1")
                        tr = rp.tile([128, 512], F32, tag="tr")
                        nc.vector.tensor_mul(t0[:], psq[:], cosrq_sb[:, sl])
                        nc.vector.tensor_mul(t1[:], psq[:], sinrq_sb[:, sl])
                        nc.sync.dma_start(out=tr[0:64, :], in_=t1[64:128, :])
                        nc.sync.dma_start(out=tr[64:128, :], in_=t1[0:64, :])
                        nc.vector.tensor_add(t0[:], t0[:], aq_sb[:, j, sl])
                        nc.vector.tensor_add(qT[:, j, sl], t0[:], tr[:])

            # ---------------- causal attention, S^T form ----------------
            with (
                tc.tile_pool(name="es", bufs=2) as esp,
                tc.tile_pool(name="rq", bufs=2) as rqp,
                tc.tile_pool(name="ps_s", bufs=2, space="PSUM") as ps_s,
                tc.tile_pool(name="ps_r", bufs=2, space="PSUM") as ps_r,
                tc.tile_pool(name="ps_a", bufs=1, space="PSUM") as ps_a,
                tc.tile_pool(name="ps_o", bufs=2, space="PSUM") as ps_o,
            ):
                for j in range(QH):
                    es = [esp.tile([128, S - 128 * k], BF16, tag=f"es{k}",
                                   name=f"es{k}")
                          for k in range(NB)]
                    rsum = rqp.tile([1, S], F32, tag="rsum")
                    for qc in range(2):
                        q0, q1 = qc * 512, (qc + 1) * 512
                        pr = ps_r.tile([1, 512], F32, tag="pr")
                        kmax = 4 * (qc + 1)
                        for k in range(kmax):
                            lo = max(q0, k * 128)
                            w = q1 - lo
                            pss = ps_s.tile([128, 512], F32, tag="pss")
                            nc.tensor.matmul(
                                pss[:, :w], kT[:, k * 128:(k + 1) * 128],
                                qT[:, j, lo:q1])
                            loc = lo - k * 128
                            nc.scalar.activation(es[k][:, loc:loc + w],
                                                 pss[:, :w], AF.Exp)
                            if lo == k * 128:  # diagonal block: causal mask
                                nc.vector.tensor_mul(es[k][:, 0:128],
                                                     es[k][:, 0:128], tri_sb[:])
                            nc.tensor.matmul(pr[:, lo - q0:lo - q0 + w],
                                             ones_sb[:], es[k][:, loc:loc + w],
                                             start=(k == 0), stop=(k == kmax - 1))
                        nc.vector.reciprocal(rsum[:, q0:q1], pr[:])
                    nc.sync.dma_start(out=scratch[j:j + 1, :], in_=rsum[:])
                    recq = rqp.tile([128, S], F32, tag="recq")
                    nc.sync.dma_start(out=recq[:],
                                      in_=bc_ap(scratch[j:j + 1, :], 128, S))
                    for half in range(2):
                        pav = ps_a.tile([128, 512], F32, tag=f"pav{half}",
                                        name=f"pav{half}")
                        for qi in range(4):
                            qb = half * 4 + qi
                            for k in range(qb + 1):
                                nc.tensor.matmul(
                                    pav[:, qi * 128:(qi + 1) * 128],
                                    vN[:, k, :],
                                    es[k][:, (qb - k) * 128:(qb - k + 1) * 128],
                                    start=(k == 0), stop=(k == qb))
                        nc.vector.tensor_mul(
                            avn[:, j, half * 512:(half + 1) * 512], pav[:],
                            recq[:, half * 512:(half + 1) * 512])

                # ---------------- partial out-proj ----------------
                with (
                    tc.tile_pool(name="wop", bufs=2) as wop,
                    tc.tile_pool(name="po", bufs=3) as pop,
                ):
                    wo_sb = wop.tile([128, QH, DT, 128], BF16)
                    for j in range(QH):
                        nc.sync.dma_start(out=wo_sb[:, j], in_=wo[j])
                    for dt_ in range(DT):
                        for qc in range(2):
                            sl = slice(qc * 512, (qc + 1) * 512)
                            pso = ps_o.tile([128, 512], F32, tag="pso")
                            for j in range(QH):
                                nc.tensor.matmul(pso[:], wo_sb[:, j, dt_, :],
                                                 avn[:, j, sl],
                                                 start=(j == 0),
                                                 stop=(j == QH - 1))
                            ot = pop.tile([128, 512], BF16, tag="ot")
                            if (dt_ + qc) % 2 == 0:
                                nc.scalar.copy(ot[:], pso[:])
                            else:
                                nc.vector.tensor_copy(ot[:], pso[:])
                            nc.sync.dma_start(out=pout[dt_, :, sl], in_=ot[:])
    nc.compile()
    return nc


def host_attn_inputs(x, cos, sin, ln1_w, w_qkv, w_out, n_cores=8):
    """Per-core input maps for build_attn. Core (b, kvh) = b*4 + kvh."""
    x = np.asarray(x, np.float32)
    wqkvT = (np.asarray(w_qkv, np.float32) * ln1_w[None, :]).T  # [D, 3072]
    wqm = wqkvT[:, :H * HD]
    wkm = wqkvT[:, H * HD:(H + KV) * HD]
    wvm = wqkvT[:, (H + KV) * HD:]
    woT = np.asarray(w_out, np.float32).T                       # [H*HD, D]
    sinp = np.asarray(sin, np.float32).copy()
    sinp[:, :HD // 2] *= -1.0
    cosT = np.ascontiguousarray(np.asarray(cos, np.float32).T)  # [128, S]
    sinT = np.ascontiguousarray(sinp.T)
    # device computes t1 = k*sinS then swaps halves; for the result at row o
    # to be k[swap(o)]*sinp[o], the multiply must use sinS = sinp o swap
    sinS = np.concatenate([sinT[64:], sinT[:64]], axis=0)
    scale = np.float32(1.0 / np.sqrt(HD))

    mu = x.mean(-1)                                             # [B, S]
    rstd = 1.0 / np.sqrt(x.var(-1) + EPS)
    nmr = -mu * rstd

    wq4 = wqm.reshape(DT, 128, H, 128)
    wk4 = wkm.reshape(DT, 128, KV, 128)
    wv4 = wvm.reshape(DT, 128, KV, 128)
    wo4 = woT.reshape(H, 128, DT, 128)
    ident = np.eye(128, dtype=BF)
    ones_in = np.ones((128, 1), BF)
    tri_in = np.triu(np.ones((128, 128), np.float32)).astype(BF)

    def rot128(v):
        return np.concatenate([v[64:], v[:64]])

    maps = []
    for b in range(B):
        xtb = np.ascontiguousarray(x[b].T).reshape(DT, 128, S).astype(BF)
        for kvh in range(KV):
            cr = cosT * rstd[b][None, :]
            sr = sinS * rstd[b][None, :]
            wks = wkm[:, kvh * 128:(kvh + 1) * 128].sum(0)      # [128]
            wvs = wvm[:, kvh * 128:(kvh + 1) * 128].sum(0)
            ak_t = nmr[b][None, :] * (wks[:, None] * cosT
                                      + rot128(wks)[:, None] * sinT)
            aq_t = np.empty((QH, 128, S), np.float32)
            for j in range(QH):
                wqs = wqm[:, (kvh * QH + j) * 128:(kvh * QH + j + 1) * 128].sum(0)
                aq_t[j] = scale * nmr[b][None, :] * (
                    wqs[:, None] * cosT + rot128(wqs)[:, None] * sinT)
            maps.append({
                "xt": xtb,
                "wk": np.ascontiguousarray(
                    wk4[:, :, kvh, :].transpose(1, 0, 2)).astype(BF),
                "wv": np.ascontiguousarray(
                    wv4[:, :, kvh, :].transpose(1, 0, 2)).astype(BF),
                "wq": np.ascontiguousarray(
                    wq4[:, :, kvh * QH:(kvh + 1) * QH, :]
                    .transpose(2, 1, 0, 3)).astype(BF),
                "wo": np.ascontiguousarray(
                    wo4[kvh * QH:(kvh + 1) * QH]).astype(BF),
                "cosr": cr.astype(BF), "sinr": sr.astype(BF),
                "cosrq": (cr * scale).astype(BF),
                "sinrq": (sr * scale).astype(BF),
                "ak": ak_t.astype(BF), "aq": aq_t.astype(BF),
                "av": (wvs[:, None] * nmr[b][None, :]).astype(BF),
                "rstdr": np.ascontiguousarray(rstd[b][None, :]),
                "tri": tri_in, "ident": ident, "ones": ones_in,
            })
    return maps


def assemble_attn(results, x):
    """Sum per-kv-head partial out-projections, add residual."""
    resid = np.asarray(x, np.float32).copy()
    for b in range(B):
        acc = np.zeros((D, S), np.float32)
        for kvh in range(KV):
            acc += results[b * KV + kvh]["pout"].reshape(D, S).astype(np.float32)
        resid[b] += acc.T
    return resid


# ======================= host routing (+ exact repair) =======================

def _ln(v, w):
    mu = v.mean(-1, keepdims=True)
    var = v.var(-1, keepdims=True)
    return (v - mu) / np.sqrt(var + EPS) * w


def _softmax(v):
    p = np.exp(v - v.max(-1, keepdims=True))
    return p / p.sum(-1, keepdims=True)


def exact_logits_rows(tokens, x, cos, sin, ln1_w, ln2_w, w_qkv, w_out,
                      w_router):
    """Reference-exact (fp32 numpy) router logits for the given token ids."""
    x = np.asarray(x, np.float32)
    h1 = _ln(x, np.asarray(ln1_w, np.float32))
    wqkvT = np.asarray(w_qkv, np.float32).T
    wqm, wkm, wvm = (wqkvT[:, :H * HD], wqkvT[:, H * HD:(H + KV) * HD],
                     wqkvT[:, (H + KV) * HD:])
    cos = np.asarray(cos, np.float32)
    sin = np.asarray(sin, np.float32)

    def rope(v, p):  # v [..., heads, HD]
        r = np.concatenate([-v[..., HD // 2:], v[..., :HD // 2]], -1)
        return v * cos[p][None, :] + r * sin[p][None, :]

    bs = [t // S for t in tokens]
    out_rows = np.empty((len(tokens), E), np.float32)
    kvc = {}
    for i, t in enumerate(tokens):
        b, p = t // S, t % S
        if b not in kvc:
            k_all = (h1[b] @ wkm).reshape(S, KV, HD)
            k_all = np.stack([rope(k_all[pp], pp) for pp in range(S)])
            v_all = (h1[b] @ wvm).reshape(S, KV, HD)
            kvc[b] = (k_all, v_all)
        k_all, v_all = kvc[b]
        q = rope((h1[b, p] @ wqm).reshape(H, HD), p)            # [H, HD]
        kk = k_all[:p + 1].repeat(QH, 1)                        # [p+1, H, HD]
        vv = v_all[:p + 1].repeat(QH, 1)
        sc = np.einsum('hd,khd->hk', q, kk) / np.sqrt(HD).astype(np.float32)
        pm = _softmax(sc)
        at = np.einsum('hk,khd->hd', pm, vv).reshape(H * HD)
        row = at @ np.asarray(w_out, np.float32).T + x[b, p]
        h2 = _ln(row[None, :], np.asarray(ln2_w, np.float32))[0]
        out_rows[i] = h2 @ np.asarray(w_router, np.float32).T
    return out_rows


def routing_from_logits(lg):
    pr = _softmax(lg)
    te = np.argsort(-pr, 1)[:, :K_TOP]
    tw = np.take_along_axis(pr, te, 1)
    tw = tw / np.abs(tw).sum(1, keepdims=True)
    flat_e = te.ravel()
    flat_t = np.repeat(np.arange(lg.shape[0]), K_TOP)
    flat_w = tw.ravel()
    assign = [flat_t[flat_e == e] for e in range(E)]
    aw = [flat_w[flat_e == e] for e in range(E)]
    return assign, aw


# ======================= MoE launch (expert parallel) =======================

def chunks(C):
    if C <= 512:
        return [(0, C)]
    if C <= 1024:
        h = (C // 2 + 31) // 32 * 32
        return [(0, h), (h, C - h)]
    return [(0, 512), (512, 512), (1024, C - 1024)]


def build_moe(C, n_cores=8):
    CH = chunks(C)
    nc = bacc.Bacc("TRN2", target_bir_lowering=False, debug=False,
                   num_devices=n_cores)
    xe = nc.dram_tensor("xe", [DT, 128, C], BF16, kind="ExternalInput").ap()
    wg = nc.dram_tensor("wg", [DT, 128, DT, 128], BF16, kind="ExternalInput").ap()
    wu = nc.dram_tensor("wu", [DT, 128, DT, 128], BF16, kind="ExternalInput").ap()
    wd = nc.dram_tensor("wd", [DT, 128, DT, 128], BF16, kind="ExternalInput").ap()
    wec = nc.dram_tensor("wec", [1, C], F32, kind="ExternalInput").ap()
    ye = nc.dram_tensor("ye", [DT, 128, C], F32, kind="ExternalOutput").ap()

    with tile.TileContext(nc) as tc:
        with (
            tc.tile_pool(name="res", bufs=1) as res,
            tc.tile_pool(name="wp", bufs=3) as wp,
            tc.tile_pool(name="sg", bufs=3) as sgp,
            tc.tile_pool(name="yo", bufs=3) as yop,
        ):
            xsb = res.tile([128, DT, C], BF16)
            for d in range(DT):
                nc.sync.dma_start(out=xsb[:, d, :], in_=xe[d])
            webc = res.tile([128, C], F32)
            nc.sync.dma_start(out=webc[:], in_=bc_ap(wec, 128, C))
            mT = res.tile([128, DT, C], BF16)

            with (
                tc.tile_pool(name="psgu", bufs=1, space="PSUM") as psg,
                tc.tile_pool(name="psy", bufs=2, space="PSUM") as psy,
            ):
                for f in range(DT):
                    pgs = [psg.tile([128, w], F32, name=f"pg{ci}", tag=f"pg{ci}")
                           for ci, (_, w) in enumerate(CH)]
                    pus = [psg.tile([128, w], F32, name=f"pu{ci}", tag=f"pu{ci}")
                           for ci, (_, w) in enumerate(CH)]
                    wgt = wp.tile([128, DT, 128], BF16, tag="wg")
                    nc.sync.dma_start(out=wgt[:], in_=wg[f])
                    wut = wp.tile([128, DT, 128], BF16, tag="wu")
                    nc.sync.dma_start(out=wut[:], in_=wu[f])
                    for d in range(DT):
                        for ci, (c0, w) in enumerate(CH):
                            nc.tensor.matmul(pgs[ci][:], wgt[:, d, :],
                                             xsb[:, d, c0:c0 + w],
                                             start=(d == 0), stop=(d == DT - 1))
                        for ci, (c0, w) in enumerate(CH):
                            nc.tensor.matmul(pus[ci][:], wut[:, d, :],
                                             xsb[:, d, c0:c0 + w],
                                             start=(d == 0), stop=(d == DT - 1))
                    for ci, (c0, w) in enumerate(CH):
                        sg = sgp.tile([128, 512], BF16, tag="sg")
                        nc.scalar.activation(sg[:, :w], pgs[ci][:], AF.Silu)
                        nc.vector.tensor_mul(mT[:, f, c0:c0 + w], sg[:, :w],
                                             pus[ci][:])

                for d2 in range(DT):
                    pys = [psy.tile([128, w], F32, name=f"py{ci}", tag=f"py{ci}")
                           for ci, (_, w) in enumerate(CH)]
                    wdt = wp.tile([128, DT, 128], BF16, tag="wd")
                    nc.sync.dma_start(out=wdt[:], in_=wd[d2])
                    for f in range(DT):
                        for ci, (c0, w) in enumerate(CH):
                            nc.tensor.matmul(pys[ci][:], wdt[:, f, :],
                                             mT[:, f, c0:c0 + w],
                                             start=(f == 0), stop=(f == DT - 1))
                    for ci, (c0, w) in enumerate(CH):
                        yt = yop.tile([128, 512], F32, tag="yt")
                        nc.vector.tensor_mul(yt[:, :w], pys[ci][:],
                                             webc[:, c0:c0 + w])
                        nc.sync.dma_start(out=ye[d2, :, c0:c0 + w], in_=yt[:, :w])
    nc.compile()
    return nc


def host_moe_inputs(h2T, assign, aw, C, w_gate, w_up, w_down):
    """h2T [D, T] fp32; per-expert token gather + bf16 weight tiling."""
    maps = []
    for e in range(E):
        n = len(assign[e])
        assert n <= C, f"expert {e} count {n} > capacity {C}"
        xeT = np.zeros((D, C), BF)
        xeT[:, :n] = h2T[:, assign[e]].astype(BF)
        wec = np.zeros((1, C), np.float32)
        wec[0, :n] = aw[e]
        maps.append({
            "xe": np.ascontiguousarray(xeT.reshape(DT, 128, C)),
            "wg": np.ascontiguousarray(
                np.asarray(w_gate[e], np.float32)
                .reshape(DT, 128, DT, 128).transpose(2, 1, 0, 3)).astype(BF),
            "wu": np.ascontiguousarray(
                np.asarray(w_up[e], np.float32)
                .reshape(DT, 128, DT, 128).transpose(2, 1, 0, 3)).astype(BF),
            "wd": np.ascontiguousarray(
                np.asarray(w_down[e], np.float32)
                .reshape(DT, 128, DT, 128).transpose(2, 1, 0, 3)).astype(BF),
            "wec": wec,
        })
    return maps


# ======================= top-level kernel =======================

_cache = {}


def kernel(hidden_states, cos, sin, ln1_w, ln2_w, w_qkv, w_out,
           w_router, w_gate, w_up, w_down):
    hidden_states = np.asarray(hidden_states, np.float32)
    ln1_w = np.asarray(ln1_w, np.float32)
    ln2_w = np.asarray(ln2_w, np.float32)

    if "attn" not in _cache:
        _cache["attn"] = build_attn()
    maps = host_attn_inputs(hidden_states, cos, sin, ln1_w, w_qkv, w_out)
    res1 = run_bass_kernel_spmd(_cache["attn"], maps, list(range(8)))
    resid = assemble_attn(res1.results, hidden_states)

    h2 = _ln(resid.reshape(-1, D), ln2_w)
    lg = h2 @ np.asarray(w_router, np.float32).T
    srt = np.sort(lg, 1)
    marked = np.flatnonzero(srt[:, -2] - srt[:, -3] < THETA)
    if len(marked):
        lg[marked] = exact_logits_rows(marked.tolist(), hidden_states, cos,
                                       sin, ln1_w, ln2_w, w_qkv, w_out,
                                       w_router)
    assign, aw = routing_from_logits(lg)
    counts = [len(a) for a in assign]
    C = max(256, (max(counts) + 31) // 32 * 32)

    if ("moe", C) not in _cache:
        _cache[("moe", C)] = build_moe(C)
    maps2 = host_moe_inputs(np.ascontiguousarray(h2.T), assign, aw, C,
                            w_gate, w_up, w_down)
    res2 = run_bass_kernel_spmd(_cache[("moe", C)], maps2, list(range(8)))

    T = B * S
    out_full = np.zeros((T, D), np.float32)
    for e in range(E):
        ye = res2.results[e]["ye"].reshape(D, C)
        out_full[assign[e]] += ye[:, :counts[e]].T

    return out_full.reshape(B, S, D), resid


# revision 13
# speedup vs baseline: 1.8467x; 1.0294x over previous
"""Self-contained Trainium2 Bass kernel for nn_DbrxBlock_40492951667588.

DBRX block: LN1 -> GQA attention (RoPE, causal) -> residual+LN2 -> top-2/8 MoE.
8 NeuronCores, two SPMD launches, all matmuls in bf16 (fp32 psum accumulate):

  launch 1 (attention, batch x kv-head sharding): core (b, kvh) projects
    K/V for its one kv head and Q for its 4 q-heads over batch b's 1024
    tokens, computes causal scores transposed (S^T = K^T tile x Q, so no
    P transposes and no masked-out compute), AV, and a partial out-proj
    over its 4 heads' dims.  LN1 is folded in host-side (rstd/shift are
    baked into rope cos/sin and additive tiles); LN2/router run on host.
  host: reduce out-proj partials + residual, LN2, router logits, top-2.
    Tokens whose top2/top3 margin is below a threshold get their logits
    recomputed exactly (numpy fp32 attention rows) so expert selection
    always matches the fp32 reference regardless of device rounding.
  launch 2 (MoE, expert parallel): core e owns expert e, capacity-padded
    token batch, bf16 gate/up/silu/down, combine weights applied on-core.
"""
import numpy as np
import ml_dtypes
import concourse.bacc as bacc
import concourse.bass as bass
import concourse.mybir as mybir
import concourse.tile as tile
from concourse.bass_utils import run_bass_kernel_spmd

F32 = mybir.dt.float32
BF16 = mybir.dt.bfloat16
AF = mybir.ActivationFunctionType
BF = ml_dtypes.bfloat16

B, S, D = 2, 1024, 2048
H, KV, HD = 16, 4, 128
E, K_TOP, F = 8, 2, 2048
DT = D // 128            # 16 d-tiles
NB = S // 128            # 8 token blocks per batch
EPS = 1e-5
QH = H // KV             # 4 q heads per kv head / core
THETA = 0.025            # top2/top3 logit margin below which host recomputes


def bc_ap(ap, parts, n):
    """Partition-broadcast read AP: [parts, n] from a [1, n] row."""
    return bass.AP(tensor=ap.tensor, offset=ap.offset, ap=[[0, parts], [1, n]])


# ======================= attention launch =======================

def build_attn(n_cores=8):
    nc = bacc.Bacc("TRN2", target_bir_lowering=False, debug=False,
                   num_devices=n_cores)
    xt = nc.dram_tensor("xt", [DT, 128, S], BF16, kind="ExternalInput").ap()
    wk = nc.dram_tensor("wk", [128, DT, 128], BF16, kind="ExternalInput").ap()
    wv = nc.dram_tensor("wv", [128, DT, 128], BF16, kind="ExternalInput").ap()
    wq = nc.dram_tensor("wq", [QH, 128, DT, 128], BF16, kind="ExternalInput").ap()
    wo = nc.dram_tensor("wo", [QH, 128, DT, 128], BF16, kind="ExternalInput").ap()
    cosr = nc.dram_tensor("cosr", [128, S], BF16, kind="ExternalInput").ap()
    sinr = nc.dram_tensor("sinr", [128, S], BF16, kind="ExternalInput").ap()
    cosrq = nc.dram_tensor("cosrq", [128, S], BF16, kind="ExternalInput").ap()
    sinrq = nc.dram_tensor("sinrq", [128, S], BF16, kind="ExternalInput").ap()
    ak = nc.dram_tensor("ak", [128, S], BF16, kind="ExternalInput").ap()
    aq = nc.dram_tensor("aq", [QH, 128, S], BF16, kind="ExternalInput").ap()
    av = nc.dram_tensor("av", [128, S], BF16, kind="ExternalInput").ap()
    rstdr = nc.dram_tensor("rstdr", [1, S], F32, kind="ExternalInput").ap()
    tri = nc.dram_tensor("tri", [128, 128], BF16, kind="ExternalInput").ap()
    ident = nc.dram_tensor("ident", [128, 128], BF16, kind="ExternalInput").ap()
    ones = nc.dram_tensor("ones", [128, 1], BF16, kind="ExternalInput").ap()

    pout = nc.dram_tensor("pout", [DT, 128, S], BF16, kind="ExternalOutput").ap()
    scratch = nc.dram_tensor("scratch", [QH, S], BF16).ap()  # denom bounce

    with tile.TileContext(nc) as tc, nc.allow_low_precision(
            reason="bf16 rope/softmax intermediates, validated vs reference"):
        with (
            tc.tile_pool(name="cst", bufs=1) as cst,
            tc.tile_pool(name="big", bufs=1) as big,
        ):
            tri_sb = cst.tile([128, 128], BF16)
            nc.sync.dma_start(out=tri_sb[:], in_=tri[:])
            id_sb = cst.tile([128, 128], BF16)
            nc.sync.dma_start(out=id_sb[:], in_=ident[:])
            ones_sb = cst.tile([128, 1], BF16)
            nc.sync.dma_start(out=ones_sb[:], in_=ones[:])
            rstd_bc = cst.tile([128, S], F32)
            nc.sync.dma_start(out=rstd_bc[:], in_=bc_ap(rstdr, 128, S))

            xts = big.tile([128, DT, S], BF16)
            for d in range(DT):
                nc.sync.dma_start(out=xts[:, d, :], in_=xt[d])
            kT = big.tile([128, S], BF16)
            vN = big.tile([128, NB, 128], BF16)      # [t-part, kv-tile, hd]
            qT = big.tile([128, QH, S], BF16)
            avn = big.tile([128, QH, S], BF16)       # normalized AV^T per head

            # ---------------- projections + rope ----------------
            with (
                tc.tile_pool(name="cs", bufs=1) as cs,
                tc.tile_pool(name="rp", bufs=2) as rp,
                tc.tile_pool(name="ps_p", bufs=2, space="PSUM") as ps_p,
                tc.tile_pool(name="ps_t", bufs=2, space="PSUM") as ps_t,
            ):
                cosr_sb = cs.tile([128, S], BF16)
                nc.sync.dma_start(out=cosr_sb[:], in_=cosr[:])
                sinr_sb = cs.tile([128, S], BF16)
                nc.sync.dma_start(out=sinr_sb[:], in_=sinr[:])
                cosrq_sb = cs.tile([128, S], BF16)
                nc.sync.dma_start(out=cosrq_sb[:], in_=cosrq[:])
                sinrq_sb = cs.tile([128, S], BF16)
                nc.sync.dma_start(out=sinrq_sb[:], in_=sinrq[:])
                ak_sb = cs.tile([128, S], BF16)
                nc.sync.dma_start(out=ak_sb[:], in_=ak[:])
                aq_sb = cs.tile([128, QH, S], BF16)
                for j in range(QH):
                    nc.sync.dma_start(out=aq_sb[:, j, :], in_=aq[j])
                av_sb = cs.tile([128, S], BF16)
                nc.sync.dma_start(out=av_sb[:], in_=av[:])
                wk_sb = cs.tile([128, DT, 128], BF16)
                nc.sync.dma_start(out=wk_sb[:], in_=wk[:])
                wv_sb = cs.tile([128, DT, 128], BF16)
                nc.sync.dma_start(out=wv_sb[:], in_=wv[:])
                wq_sb = cs.tile([128, QH, DT, 128], BF16)
                for j in range(QH):
                    nc.sync.dma_start(out=wq_sb[:, j], in_=wq[j])

                # K: k = kraw*cosr + rot(kraw)*sinr + ak
                for c in range(2):
                    sl = slice(c * 512, (c + 1) * 512)
                    psk = ps_p.tile([128, 512], F32, tag="psp")
                    for d in range(DT):
                        nc.tensor.matmul(psk[:], wk_sb[:, d, :], xts[:, d, sl],
                                         start=(d == 0), stop=(d == DT - 1))
                    t0 = rp.tile([128, 512], BF16, tag="t0")
                    t1 = rp.tile([128, 512], BF16, tag="t1")
                    tr = rp.tile([128, 512], BF16, tag="tr")
                    nc.vector.tensor_mul(t0[:], psk[:], cosr_sb[:, sl])
                    nc.vector.tensor_mul(t1[:], psk[:], sinr_sb[:, sl])
                    nc.sync.dma_start(out=tr[0:64, :], in_=t1[64:128, :])
                    nc.sync.dma_start(out=tr[64:128, :], in_=t1[0:64, :])
                    nc.vector.tensor_add(t0[:], t0[:], ak_sb[:, sl])
                    nc.vector.tensor_add(kT[:, sl], t0[:], tr[:])

                # V^T then transpose to [t, hd]
                vtt = cs.tile([128, S], BF16)
                for c in range(2):
                    sl = slice(c * 512, (c + 1) * 512)
                    psv = ps_p.tile([128, 512], F32, tag="psp")
                    for d in range(DT):
                        nc.tensor.matmul(psv[:], wv_sb[:, d, :], xts[:, d, sl],
                                         start=(d == 0), stop=(d == DT - 1))
                    t0 = rp.tile([128, 512], BF16, tag="t0")
                    nc.vector.tensor_mul(t0[:], psv[:], rstd_bc[:, sl])
                    nc.vector.tensor_add(vtt[:, sl], t0[:], av_sb[:, sl])
                for tt in range(NB):
                    ptp = ps_t.tile([128, 128], BF16, tag="ptp")
                    nc.tensor.transpose(ptp[:], vtt[:, tt * 128:(tt + 1) * 128],
                                        id_sb[:])
                    nc.scalar.copy(vN[:, tt, :], ptp[:])

                # Q (scale 1/sqrt(HD) folded into cosrq/sinrq/aq)
                for j in range(QH):
                    for c in range(2):
                        sl = slice(c * 512, (c + 1) * 512)
                        psq = ps_p.tile([128, 512], F32, tag="psp")
                        for d in range(DT):
                            nc.tensor.matmul(psq[:], wq_sb[:, j, d, :],
                                             xts[:, d, sl],
                                             start=(d == 0), stop=(d == DT - 1))
                        t0 = rp.tile([128, 512], BF16, tag="t0")
                        t1 = rp.tile([128, 512], BF16, tag="t1")
                        tr = rp.tile([128, 512], BF16, tag="tr")
                        nc.vector.tensor_mul(t0[:], psq[:], cosrq_sb[:, sl])
                        nc.vector.tensor_mul(t1[:], psq[:], sinrq_sb[:, sl])
                        nc.sync.dma_start(out=tr[0:64, :], in_=t1[64:128, :])
                        nc.sync.dma_start(out=tr[64:128, :], in_=t1[0:64, :])
                        nc.vector.tensor_add(t0[:], t0[:], aq_sb[:, j, sl])
                        nc.vector.tensor_add(qT[:, j, sl], t0[:], tr[:])

            # ---------------- causal attention, S^T form ----------------
            with (
                tc.tile_pool(name="es", bufs=2) as esp,
                tc.tile_pool(name="rq", bufs=2) as rqp,
                tc.tile_pool(name="ps_s", bufs=2, space="PSUM") as ps_s,
                tc.tile_pool(name="ps_r", bufs=2, space="PSUM") as ps_r,
                tc.tile_pool(name="ps_a", bufs=1, space="PSUM") as ps_a,
                tc.tile_pool(name="ps_o", bufs=2, space="PSUM") as ps_o,
            ):
                for j in range(QH):
                    es = [esp.tile([128, S - 128 * k], BF16, tag=f"es{k}",
                                   name=f"es{k}")
                          for k in range(NB)]
                    rsum = rqp.tile([1, S], BF16, tag="rsum")
                    for qc in range(2):
                        q0, q1 = qc * 512, (qc + 1) * 512
                        pr = ps_r.tile([1, 512], F32, tag="pr")
                        kmax = 4 * (qc + 1)
                        for k in range(kmax):
                            lo = max(q0, k * 128)
                            w = q1 - lo
                            pss = ps_s.tile([128, 512], F32, tag="pss")
                            nc.tensor.matmul(
                                pss[:, :w], kT[:, k * 128:(k + 1) * 128],
                                qT[:, j, lo:q1])
                            loc = lo - k * 128
                            nc.scalar.activation(es[k][:, loc:loc + w],
                                                 pss[:, :w], AF.Exp)
                            if lo == k * 128:  # diagonal block: causal mask
                                nc.vector.tensor_mul(es[k][:, 0:128],
                                                     es[k][:, 0:128], tri_sb[:])
                            nc.tensor.matmul(pr[:, lo - q0:lo - q0 + w],
                                             ones_sb[:], es[k][:, loc:loc + w],
                                             start=(k == 0), stop=(k == kmax - 1))
                        nc.vector.reciprocal(rsum[:, q0:q1], pr[:])
                    nc.sync.dma_start(out=scratch[j:j + 1, :], in_=rsum[:])
                    recq = rqp.tile([128, S], BF16, tag="recq")
                    nc.sync.dma_start(out=recq[:],
                                      in_=bc_ap(scratch[j:j + 1, :], 128, S))
                    for half in range(2):
                        pav = ps_a.tile([128, 512], F32, tag=f"pav{half}",
                                        name=f"pav{half}")
                        for qi in range(4):
                            qb = half * 4 + qi
                            for k in range(qb + 1):
                                nc.tensor.matmul(
                                    pav[:, qi * 128:(qi + 1) * 128],
                                    vN[:, k, :],
                                    es[k][:, (qb - k) * 128:(qb - k + 1) * 128],
                                    start=(k == 0), stop=(k == qb))
                        nc.vector.tensor_mul(
                            avn[:, j, half * 512:(half + 1) * 512], pav[:],
                            recq[:, half * 512:(half + 1) * 512])

                # ---------------- partial out-proj ----------------
                with (
                    tc.tile_pool(name="wop", bufs=2) as wop,
                    tc.tile_pool(name="po", bufs=3) as pop,
                ):
                    wo_sb = wop.tile([128, QH, DT, 128], BF16)
                    for j in range(QH):
                        nc.sync.dma_start(out=wo_sb[:, j], in_=wo[j])
                    for dt_ in range(DT):
                        for qc in range(2):
                            sl = slice(qc * 512, (qc + 1) * 512)
                            pso = ps_o.tile([128, 512], F32, tag="pso")
                            for j in range(QH):
                                nc.tensor.matmul(pso[:], wo_sb[:, j, dt_, :],
                                                 avn[:, j, sl],
                                                 start=(j == 0),
                                                 stop=(j == QH - 1))
                            ot = pop.tile([128, 512], BF16, tag="ot")
                            if (dt_ + qc) % 2 == 0:
                                nc.scalar.copy(ot[:], pso[:])
                            else:
                                nc.vector.tensor_copy(ot[:], pso[:])
                            nc.sync.dma_start(out=pout[dt_, :, sl], in_=ot[:])
    nc.compile()
    return nc


def host_attn_inputs(x, cos, sin, ln1_w, w_qkv, w_out, n_cores=8):
    """Per-core input maps for build_attn. Core (b, kvh) = b*4 + kvh."""
    x = np.asarray(x, np.float32)
    wqkvT = (np.asarray(w_qkv, np.float32) * ln1_w[None, :]).T  # [D, 3072]
    wqm = wqkvT[:, :H * HD]
    wkm = wqkvT[:, H * HD:(H + KV) * HD]
    wvm = wqkvT[:, (H + KV) * HD:]
    woT = np.asarray(w_out, np.float32).T                       # [H*HD, D]
    sinp = np.asarray(sin, np.float32).copy()
    sinp[:, :HD // 2] *= -1.0
    cosT = np.ascontiguousarray(np.asarray(cos, np.float32).T)  # [128, S]
    sinT = np.ascontiguousarray(sinp.T)
    # device computes t1 = k*sinS then swaps halves; for the result at row o
    # to be k[swap(o)]*sinp[o], the multiply must use sinS = sinp o swap
    sinS = np.concatenate([sinT[64:], sinT[:64]], axis=0)
    scale = np.float32(1.0 / np.sqrt(HD))

    mu = x.mean(-1)                                             # [B, S]
    rstd = 1.0 / np.sqrt(x.var(-1) + EPS)
    nmr = -mu * rstd

    wq4 = wqm.reshape(DT, 128, H, 128)
    wk4 = wkm.reshape(DT, 128, KV, 128)
    wv4 = wvm.reshape(DT, 128, KV, 128)
    wo4 = woT.reshape(H, 128, DT, 128)
    ident = np.eye(128, dtype=BF)
    ones_in = np.ones((128, 1), BF)
    tri_in = np.triu(np.ones((128, 128), np.float32)).astype(BF)

    def rot128(v):
        return np.concatenate([v[64:], v[:64]])

    maps = []
    for b in range(B):
        xtb = np.ascontiguousarray(x[b].T).reshape(DT, 128, S).astype(BF)
        for kvh in range(KV):
            cr = cosT * rstd[b][None, :]
            sr = sinS * rstd[b][None, :]
            wks = wkm[:, kvh * 128:(kvh + 1) * 128].sum(0)      # [128]
            wvs = wvm[:, kvh * 128:(kvh + 1) * 128].sum(0)
            ak_t = nmr[b][None, :] * (wks[:, None] * cosT
                                      + rot128(wks)[:, None] * sinT)
            aq_t = np.empty((QH, 128, S), np.float32)
            for j in range(QH):
                wqs = wqm[:, (kvh * QH + j) * 128:(kvh * QH + j + 1) * 128].sum(0)
                aq_t[j] = scale * nmr[b][None, :] * (
                    wqs[:, None] * cosT + rot128(wqs)[:, None] * sinT)
            maps.append({
                "xt": xtb,
                "wk": np.ascontiguousarray(
                    wk4[:, :, kvh, :].transpose(1, 0, 2)).astype(BF),
                "wv": np.ascontiguousarray(
                    wv4[:, :, kvh, :].transpose(1, 0, 2)).astype(BF),
                "wq": np.ascontiguousarray(
                    wq4[:, :, kvh * QH:(kvh + 1) * QH, :]
                    .transpose(2, 1, 0, 3)).astype(BF),
                "wo": np.ascontiguousarray(
                    wo4[kvh * QH:(kvh + 1) * QH]).astype(BF),
                "cosr": cr.astype(BF), "sinr": sr.astype(BF),
                "cosrq": (cr * scale).astype(BF),
                "sinrq": (sr * scale).astype(BF),
                "ak": ak_t.astype(BF), "aq": aq_t.astype(BF),
                "av": (wvs[:, None] * nmr[b][None, :]).astype(BF),
                "rstdr": np.ascontiguousarray(rstd[b][None, :]),
                "tri": tri_in, "ident": ident, "ones": ones_in,
            })
    return maps


def assemble_attn(results, x):
    """Sum per-kv-head partial out-projections, add residual."""
    resid = np.asarray(x, np.float32).copy()
    for b in range(B):
        acc = np.zeros((D, S), np.float32)
        for kvh in range(KV):
            acc += results[b * KV + kvh]["pout"].reshape(D, S).astype(np.float32)
        resid[b] += acc.T
    return resid


# ======================= host routing (+ exact repair) =======================

def _ln(v, w):
    mu = v.mean(-1, keepdims=True)
    var = v.var(-1, keepdims=True)
    return (v - mu) / np.sqrt(var + EPS) * w


def _softmax(v):
    p = np.exp(v - v.max(-1, keepdims=True))
    return p / p.sum(-1, keepdims=True)


def exact_logits_rows(tokens, x, cos, sin, ln1_w, ln2_w, w_qkv, w_out,
                      w_router):
    """Reference-exact (fp32 numpy) router logits for the given token ids."""
    x = np.asarray(x, np.float32)
    h1 = _ln(x, np.asarray(ln1_w, np.float32))
    wqkvT = np.asarray(w_qkv, np.float32).T
    wqm, wkm, wvm = (wqkvT[:, :H * HD], wqkvT[:, H * HD:(H + KV) * HD],
                     wqkvT[:, (H + KV) * HD:])
    cos = np.asarray(cos, np.float32)
    sin = np.asarray(sin, np.float32)

    def rope(v, p):  # v [..., heads, HD]
        r = np.concatenate([-v[..., HD // 2:], v[..., :HD // 2]], -1)
        return v * cos[p][None, :] + r * sin[p][None, :]

    bs = [t // S for t in tokens]
    out_rows = np.empty((len(tokens), E), np.float32)
    kvc = {}
    for i, t in enumerate(tokens):
        b, p = t // S, t % S
        if b not in kvc:
            k_all = (h1[b] @ wkm).reshape(S, KV, HD)
            k_all = np.stack([rope(k_all[pp], pp) for pp in range(S)])
            v_all = (h1[b] @ wvm).reshape(S, KV, HD)
            kvc[b] = (k_all, v_all)
        k_all, v_all = kvc[b]
        q = rope((h1[b, p] @ wqm).reshape(H, HD), p)            # [H, HD]
        kk = k_all[:p + 1].repeat(QH, 1)                        # [p+1, H, HD]
        vv = v_all[:p + 1].repeat(QH, 1)
        sc = np.einsum('hd,khd->hk', q, kk) / np.sqrt(HD).astype(np.float32)
        pm = _softmax(sc)
        at = np.einsum('hk,khd->hd', pm, vv).reshape(H * HD)
        row = at @ np.asarray(w_out, np.float32).T + x[b, p]
        h2 = _ln(row[None, :], np.asarray(ln2_w, np.float32))[0]
        out_rows[i] = h2 @ np.asarray(w_router, np.float32).T
    return out_rows


def routing_from_logits(lg):
    pr = _softmax(lg)
    te = np.argsort(-pr, 1)[:, :K_TOP]
    tw = np.take_along_axis(pr, te, 1)
    tw = tw / np.abs(tw).sum(1, keepdims=True)
    flat_e = te.ravel()
    flat_t = np.repeat(np.arange(lg.shape[0]), K_TOP)
    flat_w = tw.ravel()
    assign = [flat_t[flat_e == e] for e in range(E)]
    aw = [flat_w[flat_e == e] for e in range(E)]
    return assign, aw


# ======================= MoE launch (expert parallel) =======================

def chunks(C):
    if C <= 512:
        return [(0, C)]
    if C <= 1024:
        h = (C // 2 + 31) // 32 * 32
        return [(0, h), (h, C - h)]
    return [(0, 512), (512, 512), (1024, C - 1024)]


def build_moe(C, n_cores=8):
    CH = chunks(C)
    nc = bacc.Bacc("TRN2", target_bir_lowering=False, debug=False,
                   num_devices=n_cores)
    xe = nc.dram_tensor("xe", [DT, 128, C], BF16, kind="ExternalInput").ap()
    wg = nc.dram_tensor("wg", [DT, 128, DT, 128], BF16, kind="ExternalInput").ap()
    wu = nc.dram_tensor("wu", [DT, 128, DT, 128], BF16, kind="ExternalInput").ap()
    wd = nc.dram_tensor("wd", [DT, 128, DT, 128], BF16, kind="ExternalInput").ap()
    wec = nc.dram_tensor("wec", [1, C], F32, kind="ExternalInput").ap()
    ye = nc.dram_tensor("ye", [DT, 128, C], F32, kind="ExternalOutput").ap()

    with tile.TileContext(nc) as tc:
        with (
            tc.tile_pool(name="res", bufs=1) as res,
            tc.tile_pool(name="wp", bufs=3) as wp,
            tc.tile_pool(name="sg", bufs=3) as sgp,
            tc.tile_pool(name="yo", bufs=3) as yop,
        ):
            xsb = res.tile([128, DT, C], BF16)
            for d in range(DT):
                nc.sync.dma_start(out=xsb[:, d, :], in_=xe[d])
            webc = res.tile([128, C], F32)
            nc.sync.dma_start(out=webc[:], in_=bc_ap(wec, 128, C))
            mT = res.tile([128, DT, C], BF16)

            with (
                tc.tile_pool(name="psgu", bufs=1, space="PSUM") as psg,
                tc.tile_pool(name="psy", bufs=2, space="PSUM") as psy,
            ):
                for f in range(DT):
                    pgs = [psg.tile([128, w], F32, name=f"pg{ci}", tag=f"pg{ci}")
                           for ci, (_, w) in enumerate(CH)]
                    pus = [psg.tile([128, w], F32, name=f"pu{ci}", tag=f"pu{ci}")
                           for ci, (_, w) in enumerate(CH)]
                    wgt = wp.tile([128, DT, 128], BF16, tag="wg")
                    nc.sync.dma_start(out=wgt[:], in_=wg[f])
                    wut = wp.tile([128, DT, 128], BF16, tag="wu")
                    nc.sync.dma_start(out=wut[:], in_=wu[f])
                    for d in range(DT):
                        for ci, (c0, w) in enumerate(CH):
                            nc.tensor.matmul(pgs[ci][:], wgt[:, d, :],
                                             xsb[:, d, c0:c0 + w],
                                             start=(d == 0), stop=(d == DT - 1))
                        for ci, (c0, w) in enumerate(CH):
                            nc.tensor.matmul(pus[ci][:], wut[:, d, :],
                                             xsb[:, d, c0:c0 + w],
                                             start=(d == 0), stop=(d == DT - 1))
                    for ci, (c0, w) in enumerate(CH):
                        sg = sgp.tile([128, 512], BF16, tag="sg")
                        nc.scalar.activation(sg[:, :w], pgs[ci][:], AF.Silu)
                        nc.vector.tensor_mul(mT[:, f, c0:c0 + w], sg[:, :w],
                                             pus[ci][:])

                for d2 in range(DT):
                    pys = [psy.tile([128, w], F32, name=f"py{ci}", tag=f"py{ci}")
                           for ci, (_, w) in enumerate(CH)]
                    wdt = wp.tile([128, DT, 128], BF16, tag="wd")
                    nc.sync.dma_start(out=wdt[:], in_=wd[d2])
                    for f in range(DT):
                        for ci, (c0, w) in enumerate(CH):
                            nc.tensor.matmul(pys[ci][:], wdt[:, f, :],
                                             mT[:, f, c0:c0 + w],
                                             start=(f == 0), stop=(f == DT - 1))
                    for ci, (c0, w) in enumerate(CH):
                        yt = yop.tile([128, 512], F32, tag="yt")
                        nc.vector.tensor_mul(yt[:, :w], pys[ci][:],
                                             webc[:, c0:c0 + w])
                        nc.sync.dma_start(out=ye[d2, :, c0:c0 + w], in_=yt[:, :w])
    nc.compile()
    return nc


def host_moe_inputs(h2T, assign, aw, C, w_gate, w_up, w_down):
    """h2T [D, T] fp32; per-expert token gather + bf16 weight tiling."""
    maps = []
    for e in range(E):
        n = len(assign[e])
        assert n <= C, f"expert {e} count {n} > capacity {C}"
        xeT = np.zeros((D, C), BF)
        xeT[:, :n] = h2T[:, assign[e]].astype(BF)
        wec = np.zeros((1, C), np.float32)
        wec[0, :n] = aw[e]
        maps.append({
            "xe": np.ascontiguousarray(xeT.reshape(DT, 128, C)),
            "wg": np.ascontiguousarray(
                np.asarray(w_gate[e], np.float32)
                .reshape(DT, 128, DT, 128).transpose(2, 1, 0, 3)).astype(BF),
            "wu": np.ascontiguousarray(
                np.asarray(w_up[e], np.float32)
                .reshape(DT, 128, DT, 128).transpose(2, 1, 0, 3)).astype(BF),
            "wd": np.ascontiguousarray(
                np.asarray(w_down[e], np.float32)
                .reshape(DT, 128, DT, 128).transpose(2, 1, 0, 3)).astype(BF),
            "wec": wec,
        })
    return maps


# ======================= top-level kernel =======================

_cache = {}


def kernel(hidden_states, cos, sin, ln1_w, ln2_w, w_qkv, w_out,
           w_router, w_gate, w_up, w_down):
    hidden_states = np.asarray(hidden_states, np.float32)
    ln1_w = np.asarray(ln1_w, np.float32)
    ln2_w = np.asarray(ln2_w, np.float32)

    if "attn" not in _cache:
        _cache["attn"] = build_attn()
    maps = host_attn_inputs(hidden_states, cos, sin, ln1_w, w_qkv, w_out)
    res1 = run_bass_kernel_spmd(_cache["attn"], maps, list(range(8)))
    resid = assemble_attn(res1.results, hidden_states)

    h2 = _ln(resid.reshape(-1, D), ln2_w)
    lg = h2 @ np.asarray(w_router, np.float32).T
    srt = np.sort(lg, 1)
    marked = np.flatnonzero(srt[:, -2] - srt[:, -3] < THETA)
    if len(marked):
        lg[marked] = exact_logits_rows(marked.tolist(), hidden_states, cos,
                                       sin, ln1_w, ln2_w, w_qkv, w_out,
                                       w_router)
    assign, aw = routing_from_logits(lg)
    counts = [len(a) for a in assign]
    C = max(256, (max(counts) + 31) // 32 * 32)

    if ("moe", C) not in _cache:
        _cache[("moe", C)] = build_moe(C)
    maps2 = host_moe_inputs(np.ascontiguousarray(h2.T), assign, aw, C,
                            w_gate, w_up, w_down)
    res2 = run_bass_kernel_spmd(_cache[("moe", C)], maps2, list(range(8)))

    T = B * S
    out_full = np.zeros((T, D), np.float32)
    for e in range(E):
        ye = res2.results[e]["ye"].reshape(D, C)
        out_full[assign[e]] += ye[:, :counts[e]].T

    return out_full.reshape(B, S, D), resid
